# revision 1
# baseline (speedup 1.0000x reference)
"""Multi-label masked-gather mean loss on 8 Trainium2 NeuronCores.

reference:
    logp = log_softmax(x, -1); per_sample = -sum_t(mask*logp[i, y[i,t]])/count_i
    loss = mean(per_sample)

Identity used (count_i > 0):
    per_sample_i = logsumexp(x_i) - sum_t w[i,t] * x[i, y[i,t]],  w = mask/count
    loss = (sum_i logsumexp(x_i) + sum_{i,t} wneg[i,t] * x[i,y[i,t]]) / B
with wneg = -w. Data-parallel over the batch: 4096 rows -> 512 rows/core.

Per core the Bass kernel streams its x shard [512, 50257] f32 once from HBM
(memory-bound), computing exp + row-sum via ScalarE activation accumulate,
logsumexp per row, an indirect-DMA gather of the 8 labeled logits per row,
and reduces everything to a per-partition partial sum [128, 1].
Host sums the 8x128 partials and divides by B.
"""

import sys

sys.path.insert(0, "/opt/trn_rl_repo")

import math

import numpy as np

import concourse.bass as bass
import concourse.tile as tile
from concourse import bacc, mybir
from concourse import bass_utils

# Problem shape (hardcoded per contract)
B, C, T = 4096, 50257, 8
NCORES = 8
BL = B // NCORES  # 512 rows per core
P = 128
RB = BL // P      # 4 row blocks per core
CW = 16384        # column tile width (bf16 -> 32 KiB per partition)
GCOLS = BL * T // P      # 32: gathered elements per partition


MAXW = 17489                       # widest tile (pool slot size)


def _col_tiles(rb):
    """One tile per (DMA, ACT) piece — a tile is never simultaneously
    read by ACT and written by a later DMA (intra-tile sharing measured
    ~20% ACT slowdown). Row block 0 ramps up so ACT starts ~11us in and
    DMA (only ~1.3x ACT's rate) never falls behind after the start."""
    if rb == 0:
        widths = [2048, 4096, 6144, 9216, 12288, 16465]
    else:
        widths = [16384, 16384, 17489]
    tiles = []
    c0 = 0
    for w in widths:
        tiles.append((c0, w))
        c0 += w
    assert c0 == C
    return tiles


_NCT_BY_RB = [len(_col_tiles(rb)) for rb in range(RB)]
ACC_COLS = sum(_NCT_BY_RB)         # per-ACT-piece sumexp cols
OUT_COLS = ACC_COLS + 1            # + gather-dot col

_f32 = mybir.dt.float32
_bf16 = mybir.dt.bfloat16
_i32 = mybir.dt.int32

_compiled = None  # (nc, names) cache


def _build():
    nc = bacc.Bacc(
        "TRN2",
        target_bir_lowering=False,
        debug=False,
        enable_asserts=False,
        num_devices=NCORES,
    )
    x_t = nc.dram_tensor("x", [BL, C], _bf16, kind="ExternalInput")
    idx_t = nc.dram_tensor("idx", [P, GCOLS], _i32, kind="ExternalInput")
    wneg_t = nc.dram_tensor("wneg", [P, GCOLS], _f32, kind="ExternalInput")
    # cols 0..ACC_COLS-1: per-(rowblock, coltile) sumexp partials;
    # col ACC_COLS: sum_t wneg*gathered. Host sums + logs.
    out_t = nc.dram_tensor("out", [P, OUT_COLS], _f32, kind="ExternalOutput")

    x = x_t.ap()
    idx = idx_t.ap()
    wneg = wneg_t.ap()
    out = out_t.ap()

    with tile.TileContext(nc) as tc:
        with (
            tc.tile_pool(name="xin", bufs=5) as xin_pool,
            tc.tile_pool(name="scratch", bufs=1) as scratch_pool,
            tc.tile_pool(name="stats", bufs=1) as stats_pool,
            tc.tile_pool(name="gather", bufs=1) as gather_pool,
        ):
            # all partials end up here and go out in one DMA
            acc = stats_pool.tile([P, OUT_COLS], _f32)
            # self-made zero bias for Exp: avoids the const-AP preamble load
            bias0 = stats_pool.tile([P, 1], _f32)
            nc.gpsimd.memset(bias0[:], 0.0)

            # exp output scratch: values are unused, only accum_out matters
            # (fp8 keeps it small; the accumulator itself is fp32).
            exp_scratch = scratch_pool.tile([P, MAXW], mybir.dt.float8e4)

            # --- main stream: exp + row-sum via ACT activation accumulate ---
            col = 0
            for rb in range(RB):
                rows = slice(rb * P, (rb + 1) * P)
                for c0, cw in _col_tiles(rb):
                    xt = xin_pool.tile([P, MAXW], _bf16, tag="xt")
                    nc.sync.dma_start(
                        out=xt[:, :cw], in_=x[rows, c0 : c0 + cw]
                    )
                    nc.scalar.activation(
                        out=exp_scratch[:, :cw],
                        in_=xt[:, :cw],
                        func=mybir.ActivationFunctionType.Exp,
                        bias=bias0[:, 0:1],
                        accum_out=acc[:, col : col + 1],
                    )
                    col += 1
            assert col == ACC_COLS

            # --- gather path (tiny; runs in the shadow of the stream on
            # SWDGE/DVE, completes well before the final ACT) ---
            idx_tile = gather_pool.tile([P, GCOLS], _i32)
            nc.gpsimd.dma_start(out=idx_tile[:], in_=idx[:])
            w_tile = gather_pool.tile([P, GCOLS], _f32)
            nc.gpsimd.dma_start(out=w_tile[:], in_=wneg[:])
            g_tile = gather_pool.tile([P, GCOLS], _bf16)
            nc.gpsimd.indirect_dma_start(
                out=g_tile[:],
                out_offset=None,
                in_=x[:],
                in_offset=bass.IndirectOffsetOnAxis(ap=idx_tile[:], axis=1),
            )
            g32 = gather_pool.tile([P, GCOLS], _f32)
            nc.vector.tensor_copy(out=g32[:], in_=g_tile[:])
            gw = gather_pool.tile([P, GCOLS], _f32)
            nc.vector.tensor_tensor(
                out=gw[:], in0=g32[:], in1=w_tile[:], op=mybir.AluOpType.mult
            )
            nc.vector.tensor_reduce(
                out=acc[:, ACC_COLS : ACC_COLS + 1],
                in_=gw[:],
                axis=mybir.AxisListType.X,
                op=mybir.AluOpType.add,
            )

            # out via the scalar engine's HWDGE ring: no cross-engine hop
            # after the last ACT writes its accumulator column.
            nc.scalar.dma_start(out=out[:], in_=acc[:])

    nc.compile()
    return nc


def _get_compiled():
    global _compiled
    if _compiled is None:
        _compiled = _build()
    return _compiled


def _make_in_maps(x, y):
    import ml_dtypes

    # bf16 staging: halves HBM traffic; loss rel err impact ~1e-6 (rounding
    # averages out across 50k-element rows).
    x = np.ascontiguousarray(np.asarray(x, dtype=np.float32).astype(ml_dtypes.bfloat16))
    y = np.asarray(y)
    mask = y != -1
    cnt = mask.sum(axis=1)
    # rows with count 0 would be NaN in the reference; inputs never hit this
    w = np.where(mask, 1.0 / np.maximum(cnt, 1)[:, None], 0.0).astype(np.float32)
    wneg = -w
    safe = np.where(mask, y, 0).astype(np.int64)

    in_maps = []
    for m in range(NCORES):
        sl = slice(m * BL, (m + 1) * BL)
        xs = x[sl]
        flat = (
            np.arange(BL, dtype=np.int64)[:, None] * C + safe[sl]
        ).astype(np.int32)
        in_maps.append(
            {
                "x": xs,
                "idx": np.ascontiguousarray(flat.reshape(P, GCOLS)),
                "wneg": np.ascontiguousarray(wneg[sl].reshape(P, GCOLS)),
            }
        )
    return in_maps


def kernel(**inputs) -> np.ndarray:
    x, y = inputs["x"], inputs["y"]
    nc = _get_compiled()
    in_maps = _make_in_maps(x, y)
    res = bass_utils.run_bass_kernel_spmd(
        nc, in_maps, core_ids=list(range(NCORES))
    )
    total = 0.0
    for r in res.results:
        out = np.asarray(r["out"], dtype=np.float64)  # [P, OUT_COLS]
        col = 0
        for rb in range(RB):
            n = _NCT_BY_RB[rb]
            se = out[:, col : col + n].sum(axis=1)  # per-row sumexp
            total += np.log(se).sum()
            col += n
        total += out[:, ACC_COLS].sum()
    return np.float32(total / B)



# revision 4
# speedup vs baseline: 1.6419x; 1.6419x over previous
"""Multi-label masked-gather mean loss on 8 Trainium2 NeuronCores.

reference:
    logp = log_softmax(x, -1); per_sample = -sum_t(mask*logp[i, y[i,t]])/count_i
    loss = mean(per_sample)

Identity used (count_i > 0):
    per_sample_i = logsumexp(x_i) - sum_t w[i,t] * x[i, y[i,t]],  w = mask/count
    loss = (sum_i logsumexp(x_i) + sum_{i,t} wneg[i,t] * x[i,y[i,t]]) / B
with wneg = -w. Data-parallel over the batch: 4096 rows -> 512 rows/core.

v2: the exp+row-sum over the core's [512, 50257] shard is split across
engines so the scalar engine stops being the lone bottleneck:
  * ACT path (cols [0, CA)): row-major fp8 tiles, native Exp with fused
    accum_out per 128-row block (as v1, but fp8 halves DMA bytes).
  * DVE+PE path (cols [CA, C)): TRANSPOSED fp8 tiles [128 cols, 512 rows].
    DVE computes Schraudolph's bit-trick exp: int16(x*184.665 + B) viewed
    as bf16 IS approximately exp(x) (B tuned so the mean multiplicative
    bias over the fractional-exponent distribution is ~0). PE then sums
    along partitions (=columns) via an accumulating ones-matmul into
    PSUM[1, 512] = per-row partial sums.
Per-element exp error ~2-4% is iid across ~25k elements per row-half, so
row sumexp error is <<0.1%; loss tolerance is 2e-2.

Host sums ACT partials + DVE partials per row, takes log, adds the
gathered-label term, divides by B.
"""

import sys

sys.path.insert(0, "/opt/trn_rl_repo")

import math

import numpy as np

import concourse.bass as bass
import concourse.tile as tile
from concourse import bacc, mybir
from concourse import bass_utils

# Problem shape (hardcoded per contract)
B, C, T = 4096, 50257, 8
NCORES = 8
BL = B // NCORES  # 512 rows per core
P = 128
RB = BL // P      # 4 row blocks per core
GCOLS = BL * T // P      # 32: gathered elements per partition

# Column split: last N_CHUNK*128 columns go to the DVE+PE path (transposed
# layout), the first CA to the ACT path. Balanced so ACT time (~0.88ns/col
# over 4 row blocks) matches DVE convert time (~1.04ns/col).
N_CHUNK = 176
CT = N_CHUNK * P         # 22528 transposed cols
CA = C - CT              # 27729 ACT cols

# ACT path tile widths per row block (sum = CA). First tile shorter so ACT
# starts quickly; fp8 DMA runs ~2.4x ACT's rate so no long ramp needed.
ACT_WIDTHS = [4096, 11776, 11857]
assert sum(ACT_WIDTHS) == CA
MAXW = max(ACT_WIDTHS)
NT_ACT = len(ACT_WIDTHS)

ACC_COLS = RB * NT_ACT         # per-(rowblock, coltile) sumexp partials
OUT_COLS = ACC_COLS + 1        # + gather-dot col

# DVE+PE path geometry
K_SLAB = 16                    # chunks per DMA slab
N_SLAB = N_CHUNK // K_SLAB     # 11 slabs
assert N_CHUNK % K_SLAB == 0
K_CONV = 4                     # chunks per DVE convert instruction
SLAB_F = K_SLAB * BL           # slab free size (8192)
CONV_F = K_CONV * BL           # convert free size (2048)

# Schraudolph constants for bf16 bit patterns:
#   bits = x * 128*log2(e) + 128*(127 - c),  c = 0.0564298 zeroes the mean
#   multiplicative bias of the linear-mantissa approximation for f~U[0,1).
SCH_A = 128.0 * math.log2(math.e)          # 184.6650
SCH_B = 128.0 * (127.0 - 0.0564298)        # 16248.777

_f32 = mybir.dt.float32
_bf16 = mybir.dt.bfloat16
_fp8 = mybir.dt.float8e4
_i16 = mybir.dt.int16
_i32 = mybir.dt.int32

_compiled = None  # (nc, names) cache


def _build():
    nc = bacc.Bacc(
        "TRN2",
        target_bir_lowering=False,
        debug=False,
        enable_asserts=False,
        num_devices=NCORES,
    )
    x_t = nc.dram_tensor("x", [BL, C], _fp8, kind="ExternalInput")
    xt_t = nc.dram_tensor("xt", [CT, BL], _fp8, kind="ExternalInput")
    idx_t = nc.dram_tensor("idx", [P, GCOLS], _i32, kind="ExternalInput")
    wneg_t = nc.dram_tensor("wneg", [P, GCOLS], _f32, kind="ExternalInput")
    # cols 0..ACC_COLS-1: ACT-path sumexp partials; col ACC_COLS: gather dot
    out_t = nc.dram_tensor("out", [P, OUT_COLS], _f32, kind="ExternalOutput")
    # DVE+PE path per-row partial sumexp
    dve_t = nc.dram_tensor("dve", [1, BL], _f32, kind="ExternalOutput")

    x = x_t.ap()
    xt = xt_t.ap()
    idx = idx_t.ap()
    wneg = wneg_t.ap()
    out = out_t.ap()
    dve = dve_t.ap()

    with tile.TileContext(nc) as tc:
        with (
            tc.tile_pool(name="xin", bufs=4) as xin_pool,
            tc.tile_pool(name="tin", bufs=3) as tin_pool,
            tc.tile_pool(name="tconv", bufs=3) as tconv_pool,
            tc.tile_pool(name="scratch", bufs=1) as scratch_pool,
            tc.tile_pool(name="stats", bufs=1) as stats_pool,
            tc.tile_pool(name="gather", bufs=1) as gather_pool,
            tc.psum_pool(name="psum", bufs=1) as psum_pool,
        ):
            # all ACT partials end up here and go out in one DMA
            acc = stats_pool.tile([P, OUT_COLS], _f32)
            # self-made zero bias for Exp: avoids the const-AP preamble load
            bias0 = stats_pool.tile([P, 1], _f32)
            nc.gpsimd.memset(bias0[:], 0.0)
            # ones weights for the PE partition-sum
            ones_w = stats_pool.tile([P, 1], _bf16)
            nc.gpsimd.memset(ones_w[:], 1.0)

            # exp output scratch: values are unused, only accum_out matters
            exp_scratch = scratch_pool.tile([P, MAXW], mybir.dt.float8e4)

            # PE accumulates per-row sums here across all chunks
            prow = psum_pool.tile([1, BL], _f32)

            # --- issue the first ACT tile so ACT starts ASAP ---
            def act_tile(rb, ti, c0, cw):
                rows = slice(rb * P, (rb + 1) * P)
                xtile = xin_pool.tile([P, MAXW], _fp8, tag="xt")
                nc.sync.dma_start(out=xtile[:, :cw], in_=x[rows, c0 : c0 + cw])
                nc.scalar.activation(
                    out=exp_scratch[:, :cw],
                    in_=xtile[:, :cw],
                    func=mybir.ActivationFunctionType.Exp,
                    bias=bias0[:, 0:1],
                    accum_out=acc[:, rb * NT_ACT + ti : rb * NT_ACT + ti + 1],
                )

            def dve_slab(s):
                # slab DMA: chunks j=0..K_SLAB-1, chunk j covers transposed
                # rows [s*SLAB + j*128 + p], free = 512 rows
                c0 = s * K_SLAB * P
                src = xt[c0 : c0 + K_SLAB * P, :].rearrange(
                    "(j p) e -> p j e", p=P
                )
                tin = tin_pool.tile([P, SLAB_F], _fp8, tag="tin")
                dst = tin[:].rearrange("p (j e) -> p j e", e=BL)
                nc.sync.dma_start(out=dst, in_=src)
                tcv = tconv_pool.tile([P, SLAB_F], _i16, tag="tconv")
                for ci in range(K_SLAB // K_CONV):
                    f0 = ci * CONV_F
                    nc.vector.tensor_scalar(
                        out=tcv[:, f0 : f0 + CONV_F],
                        in0=tin[:, f0 : f0 + CONV_F],
                        scalar1=SCH_A,
                        scalar2=SCH_B,
                        op0=mybir.AluOpType.mult,
                        op1=mybir.AluOpType.add,
                    )
                    ebits = tcv[:, f0 : f0 + CONV_F].bitcast(_bf16)
                    for j in range(K_CONV):
                        chunk = s * K_SLAB + ci * K_CONV + j
                        nc.tensor.matmul(
                            out=prow[0:1, :],
                            lhsT=ones_w[:],
                            rhs=ebits[:, j * BL : (j + 1) * BL],
                            start=(chunk == 0),
                            stop=(chunk == N_CHUNK - 1),
                        )

            # Interleave issuance: ACT rb0 ramp first, then alternate slabs
            # with remaining ACT tiles so both DMA streams stay ahead.
            def col_starts():
                st = []
                c0 = 0
                for w in ACT_WIDTHS:
                    st.append(c0)
                    c0 += w
                return st

            starts = col_starts()
            act_list = [
                (rb, ti, starts[ti], ACT_WIDTHS[ti])
                for rb in range(RB)
                for ti in range(NT_ACT)
            ]
            # issue order: 2 ACT tiles up front, then round-robin
            order = []
            ai, si = 0, 0
            order.append(("A", act_list[0]))
            ai = 1
            while ai < len(act_list) or si < N_SLAB:
                if si < N_SLAB:
                    order.append(("S", si))
                    si += 1
                if ai < len(act_list):
                    order.append(("A", act_list[ai]))
                    ai += 1
            for kind, v in order:
                if kind == "A":
                    act_tile(*v)
                else:
                    dve_slab(v)

            # --- gather path (tiny; SWDGE + a few vector ops) ---
            idx_tile = gather_pool.tile([P, GCOLS], _i32)
            nc.gpsimd.dma_start(out=idx_tile[:], in_=idx[:])
            w_tile = gather_pool.tile([P, GCOLS], _f32)
            nc.gpsimd.dma_start(out=w_tile[:], in_=wneg[:])
            g_tile = gather_pool.tile([P, GCOLS], _fp8)
            nc.gpsimd.indirect_dma_start(
                out=g_tile[:],
                out_offset=None,
                in_=x[:],
                in_offset=bass.IndirectOffsetOnAxis(ap=idx_tile[:], axis=1),
            )
            g32 = gather_pool.tile([P, GCOLS], _f32)
            nc.vector.tensor_copy(out=g32[:], in_=g_tile[:])
            gw = gather_pool.tile([P, GCOLS], _f32)
            nc.vector.tensor_tensor(
                out=gw[:], in0=g32[:], in1=w_tile[:], op=mybir.AluOpType.mult
            )
            nc.vector.tensor_reduce(
                out=acc[:, ACC_COLS : ACC_COLS + 1],
                in_=gw[:],
                axis=mybir.AxisListType.X,
                op=mybir.AluOpType.add,
            )

            # PSUM -> SBUF -> DRAM for the DVE per-row partials
            drow = stats_pool.tile([1, BL], _f32)
            nc.vector.tensor_copy(out=drow[:], in_=prow[0:1, :])
            nc.sync.dma_start(out=dve[:], in_=drow[:])

            # out via the scalar engine's HWDGE ring: no cross-engine hop
            # after the last ACT writes its accumulator column.
            nc.scalar.dma_start(out=out[:], in_=acc[:])

    nc.compile()
    return nc


def _get_compiled():
    global _compiled
    if _compiled is None:
        _compiled = _build()
    return _compiled


def _make_in_maps(x, y):
    import ml_dtypes

    fp8 = ml_dtypes.float8_e4m3
    x = np.asarray(x, dtype=np.float32)
    y = np.asarray(y)
    mask = y != -1
    cnt = mask.sum(axis=1)
    # rows with count 0 would be NaN in the reference; inputs never hit this
    w = np.where(mask, 1.0 / np.maximum(cnt, 1)[:, None], 0.0).astype(np.float32)
    wneg = -w
    safe = np.where(mask, y, 0).astype(np.int64)

    in_maps = []
    for m in range(NCORES):
        sl = slice(m * BL, (m + 1) * BL)
        xs = np.ascontiguousarray(x[sl].astype(fp8))
        xts = np.ascontiguousarray(x[sl, CA:].T.astype(fp8))
        flat = (
            np.arange(BL, dtype=np.int64)[:, None] * C + safe[sl]
        ).astype(np.int32)
        in_maps.append(
            {
                "x": xs,
                "xt": xts,
                "idx": np.ascontiguousarray(flat.reshape(P, GCOLS)),
                "wneg": np.ascontiguousarray(wneg[sl].reshape(P, GCOLS)),
            }
        )
    return in_maps


def kernel(**inputs) -> np.ndarray:
    x, y = inputs["x"], inputs["y"]
    nc = _get_compiled()
    in_maps = _make_in_maps(x, y)
    res = bass_utils.run_bass_kernel_spmd(
        nc, in_maps, core_ids=list(range(NCORES))
    )
    total = 0.0
    for r in res.results:
        out = np.asarray(r["out"], dtype=np.float64)  # [P, OUT_COLS]
        drow = np.asarray(r["dve"], dtype=np.float64).reshape(BL)  # [BL]
        # per-row sumexp: ACT partials (3 cols per row block) + DVE partial
        for rb in range(RB):
            se = out[:, rb * NT_ACT : (rb + 1) * NT_ACT].sum(axis=1)
            se = se + drow[rb * P : (rb + 1) * P]
            total += np.log(se).sum()
        total += out[:, ACC_COLS].sum()
    return np.float32(total / B)


# revision 7
# speedup vs baseline: 1.8739x; 1.1413x over previous
"""Multi-label masked-gather mean loss on 8 Trainium2 NeuronCores.

reference:
    logp = log_softmax(x, -1); per_sample = -sum_t(mask*logp[i, y[i,t]])/count_i
    loss = mean(per_sample)

Identity used (count_i > 0):
    per_sample_i = logsumexp(x_i) - sum_t w[i,t] * x[i, y[i,t]],  w = mask/count
    loss = (sum_i logsumexp(x_i) + sum_{i,t} wneg[i,t] * x[i,y[i,t]]) / B
with wneg = -w. Data-parallel over the batch: 4096 rows -> 512 rows/core.

v2: the exp+row-sum over the core's [512, 50257] shard is split across
engines so the scalar engine stops being the lone bottleneck:
  * ACT path (cols [0, CA)): row-major fp8 tiles, native Exp with fused
    accum_out per 128-row block (as v1, but fp8 halves DMA bytes).
  * DVE+PE path (cols [CA, C)): TRANSPOSED fp8 tiles [128 cols, 512 rows].
    DVE computes Schraudolph's bit-trick exp: int16(x*184.665 + B) viewed
    as bf16 IS approximately exp(x) (B tuned so the mean multiplicative
    bias over the fractional-exponent distribution is ~0). PE then sums
    along partitions (=columns) via an accumulating ones-matmul into
    PSUM[1, 512] = per-row partial sums.
Per-element exp error ~2-4% is iid across ~25k elements per row-half, so
row sumexp error is <<0.1%; loss tolerance is 2e-2.

Host sums ACT partials + DVE partials per row, takes log, adds the
gathered-label term, divides by B.
"""

import sys

sys.path.insert(0, "/opt/trn_rl_repo")

import math

import numpy as np

import concourse.bass as bass
import concourse.tile as tile
from concourse import bacc, mybir
from concourse import bass_utils

# Problem shape (hardcoded per contract)
B, C, T = 4096, 50257, 8
NCORES = 8
BL = B // NCORES  # 512 rows per core
P = 128
RB = BL // P      # 4 row blocks per core
GCOLS = BL * T // P      # 32: gathered elements per partition

# Column split: last N_CHUNK*128 columns go to the DVE+PE path (transposed
# layout), the first CA to the ACT path. Measured rates: ACT 0.897 ns/col,
# DVE convert 0.566 ns/col (the fp8->int16 convert hits the 2x path on HW);
# balance puts ~39% on ACT, both landing just under the fp8 DMA roofline.
N_CHUNK = 240
CT = N_CHUNK * P         # 30720 transposed cols
CA = C - CT              # 19537 ACT cols

# ACT path tile widths per row block (sum = CA). First tile shorter so ACT
# starts quickly; fp8 DMA runs ~2.4x ACT's rate so no long ramp needed.
ACT_WIDTHS = [4096, 7720, 7721]
assert sum(ACT_WIDTHS) == CA
MAXW = max(ACT_WIDTHS)
NT_ACT = len(ACT_WIDTHS)

ACC_COLS = RB * NT_ACT         # per-(rowblock, coltile) sumexp partials
OUT_COLS = ACC_COLS + 1        # + gather-dot col

# DVE+PE path geometry
K_SLAB = 16                    # chunks per DMA slab
N_SLAB = N_CHUNK // K_SLAB     # 11 slabs
assert N_CHUNK % K_SLAB == 0
K_CONV = 4                     # chunks per DVE convert instruction
SLAB_F = K_SLAB * BL           # slab free size (8192)
CONV_F = K_CONV * BL           # convert free size (2048)

# Schraudolph constants for bf16 bit patterns:
#   bits = x * 128*log2(e) + 128*(127 - c),  c = 0.0564298 zeroes the mean
#   multiplicative bias of the linear-mantissa approximation for f~U[0,1).
SCH_A = 128.0 * math.log2(math.e)          # 184.6650
SCH_B = 128.0 * (127.0 - 0.0564298)        # 16248.777

_f32 = mybir.dt.float32
_bf16 = mybir.dt.bfloat16
_fp8 = mybir.dt.float8e4
_i16 = mybir.dt.int16
_i32 = mybir.dt.int32

_compiled = None  # (nc, names) cache


def _build():
    nc = bacc.Bacc(
        "TRN2",
        target_bir_lowering=False,
        debug=False,
        enable_asserts=False,
        num_devices=NCORES,
    )
    x_t = nc.dram_tensor("x", [BL, C], _fp8, kind="ExternalInput")
    xt_t = nc.dram_tensor("xt", [CT, BL], _fp8, kind="ExternalInput")
    idx_t = nc.dram_tensor("idx", [P, GCOLS], _i32, kind="ExternalInput")
    wneg_t = nc.dram_tensor("wneg", [P, GCOLS], _f32, kind="ExternalInput")
    # cols 0..ACC_COLS-1: ACT-path sumexp partials; col ACC_COLS: gather dot
    out_t = nc.dram_tensor("out", [P, OUT_COLS], _f32, kind="ExternalOutput")
    # DVE+PE path per-row partial sumexp
    dve_t = nc.dram_tensor("dve", [1, BL], _f32, kind="ExternalOutput")

    x = x_t.ap()
    xt = xt_t.ap()
    idx = idx_t.ap()
    wneg = wneg_t.ap()
    out = out_t.ap()
    dve = dve_t.ap()

    with tile.TileContext(nc) as tc:
        with (
            tc.tile_pool(name="xin", bufs=4) as xin_pool,
            tc.tile_pool(name="tin", bufs=3) as tin_pool,
            tc.tile_pool(name="tconv", bufs=3) as tconv_pool,
            tc.tile_pool(name="scratch", bufs=1) as scratch_pool,
            tc.tile_pool(name="stats", bufs=1) as stats_pool,
            tc.tile_pool(name="gather", bufs=1) as gather_pool,
            tc.psum_pool(name="psum", bufs=1) as psum_pool,
        ):
            # all ACT partials end up here and go out in one DMA
            acc = stats_pool.tile([P, OUT_COLS], _f32)
            # self-made zero bias for Exp. The scalar engine zeroes it itself
            # (memzero lowers to a Copy-activation, float bias allowed): the
            # gpsimd memset used before didn't run until ~6us in and stalled
            # ACT's first Exp until 11.4us.
            bias0 = stats_pool.tile([P, 1], _f32)
            nc.scalar.memzero(bias0[:])
            # ones weights for the PE partition-sum (DVE is idle early)
            ones_w = stats_pool.tile([P, 1], _bf16)
            nc.vector.memset(ones_w[:], 1.0)

            # exp output scratch: values are unused, only accum_out matters
            exp_scratch = scratch_pool.tile([P, MAXW], mybir.dt.float8e4)

            # PE accumulates per-row sums here across all chunks
            prow = psum_pool.tile([1, BL], _f32)

            # --- issue the first ACT tile so ACT starts ASAP ---
            def act_tile(rb, ti, c0, cw):
                rows = slice(rb * P, (rb + 1) * P)
                xtile = xin_pool.tile([P, MAXW], _fp8, tag="xt")
                nc.sync.dma_start(out=xtile[:, :cw], in_=x[rows, c0 : c0 + cw])
                nc.scalar.activation(
                    out=exp_scratch[:, :cw],
                    in_=xtile[:, :cw],
                    func=mybir.ActivationFunctionType.Exp,
                    bias=bias0[:, 0:1],
                    accum_out=acc[:, rb * NT_ACT + ti : rb * NT_ACT + ti + 1],
                )

            def dve_slab(s):
                # slab DMA: chunks j=0..K_SLAB-1, chunk j covers transposed
                # rows [s*SLAB + j*128 + p], free = 512 rows
                c0 = s * K_SLAB * P
                src = xt[c0 : c0 + K_SLAB * P, :].rearrange(
                    "(j p) e -> p j e", p=P
                )
                tin = tin_pool.tile([P, SLAB_F], _fp8, tag="tin")
                dst = tin[:].rearrange("p (j e) -> p j e", e=BL)
                nc.sync.dma_start(out=dst, in_=src)
                tcv = tconv_pool.tile([P, SLAB_F], _i16, tag="tconv")
                for ci in range(K_SLAB // K_CONV):
                    f0 = ci * CONV_F
                    nc.vector.tensor_scalar(
                        out=tcv[:, f0 : f0 + CONV_F],
                        in0=tin[:, f0 : f0 + CONV_F],
                        scalar1=SCH_A,
                        scalar2=SCH_B,
                        op0=mybir.AluOpType.mult,
                        op1=mybir.AluOpType.add,
                    )
                    ebits = tcv[:, f0 : f0 + CONV_F].bitcast(_bf16)
                    for j in range(K_CONV):
                        chunk = s * K_SLAB + ci * K_CONV + j
                        nc.tensor.matmul(
                            out=prow[0:1, :],
                            lhsT=ones_w[:],
                            rhs=ebits[:, j * BL : (j + 1) * BL],
                            start=(chunk == 0),
                            stop=(chunk == N_CHUNK - 1),
                        )

            # Interleave issuance: ACT rb0 ramp first, then alternate slabs
            # with remaining ACT tiles so both DMA streams stay ahead.
            def col_starts():
                st = []
                c0 = 0
                for w in ACT_WIDTHS:
                    st.append(c0)
                    c0 += w
                return st

            starts = col_starts()
            act_list = [
                (rb, ti, starts[ti], ACT_WIDTHS[ti])
                for rb in range(RB)
                for ti in range(NT_ACT)
            ]
            # issue order: 2 ACT tiles up front, then round-robin
            order = []
            ai, si = 0, 0
            order.append(("A", act_list[0]))
            ai = 1
            while ai < len(act_list) or si < N_SLAB:
                if si < N_SLAB:
                    order.append(("S", si))
                    si += 1
                if ai < len(act_list):
                    order.append(("A", act_list[ai]))
                    ai += 1
            # --- gather path DMAs (tiny; SWDGE on gpsimd, no deps) ---
            idx_tile = gather_pool.tile([P, GCOLS], _i32)
            nc.gpsimd.dma_start(out=idx_tile[:], in_=idx[:])
            w_tile = gather_pool.tile([P, GCOLS], _f32)
            nc.gpsimd.dma_start(out=w_tile[:], in_=wneg[:])
            g_tile = gather_pool.tile([P, GCOLS], _fp8)
            nc.gpsimd.indirect_dma_start(
                out=g_tile[:],
                out_offset=None,
                in_=x[:],
                in_offset=bass.IndirectOffsetOnAxis(ap=idx_tile[:], axis=1),
            )

            def gather_compute():
                # ~1.3us of DVE work; run it early (after slab 0's converts)
                # so the final out DMA doesn't wait on it at the tail.
                g32 = gather_pool.tile([P, GCOLS], _f32)
                nc.vector.tensor_copy(out=g32[:], in_=g_tile[:])
                gw = gather_pool.tile([P, GCOLS], _f32)
                nc.vector.tensor_tensor(
                    out=gw[:], in0=g32[:], in1=w_tile[:], op=mybir.AluOpType.mult
                )
                nc.vector.tensor_reduce(
                    out=acc[:, ACC_COLS : ACC_COLS + 1],
                    in_=gw[:],
                    axis=mybir.AxisListType.X,
                    op=mybir.AluOpType.add,
                )

            done_gather = False
            for kind, v in order:
                if kind == "A":
                    act_tile(*v)
                else:
                    dve_slab(v)
                    if not done_gather:
                        gather_compute()
                        done_gather = True

            # PSUM -> SBUF -> DRAM for the DVE per-row partials
            drow = stats_pool.tile([1, BL], _f32)
            nc.vector.tensor_copy(out=drow[:], in_=prow[0:1, :])
            nc.sync.dma_start(out=dve[:], in_=drow[:])

            # out via the scalar engine's HWDGE ring: no cross-engine hop
            # after the last ACT writes its accumulator column.
            nc.scalar.dma_start(out=out[:], in_=acc[:])

    nc.compile()
    return nc


def _get_compiled():
    global _compiled
    if _compiled is None:
        _compiled = _build()
    return _compiled


def _make_in_maps(x, y):
    import ml_dtypes

    fp8 = ml_dtypes.float8_e4m3
    x = np.asarray(x, dtype=np.float32)
    y = np.asarray(y)
    mask = y != -1
    cnt = mask.sum(axis=1)
    # rows with count 0 would be NaN in the reference; inputs never hit this
    w = np.where(mask, 1.0 / np.maximum(cnt, 1)[:, None], 0.0).astype(np.float32)
    wneg = -w
    safe = np.where(mask, y, 0).astype(np.int64)

    in_maps = []
    for m in range(NCORES):
        sl = slice(m * BL, (m + 1) * BL)
        xs = np.ascontiguousarray(x[sl].astype(fp8))
        xts = np.ascontiguousarray(x[sl, CA:].T.astype(fp8))
        flat = (
            np.arange(BL, dtype=np.int64)[:, None] * C + safe[sl]
        ).astype(np.int32)
        in_maps.append(
            {
                "x": xs,
                "xt": xts,
                "idx": np.ascontiguousarray(flat.reshape(P, GCOLS)),
                "wneg": np.ascontiguousarray(wneg[sl].reshape(P, GCOLS)),
            }
        )
    return in_maps


def kernel(**inputs) -> np.ndarray:
    x, y = inputs["x"], inputs["y"]
    nc = _get_compiled()
    in_maps = _make_in_maps(x, y)
    res = bass_utils.run_bass_kernel_spmd(
        nc, in_maps, core_ids=list(range(NCORES))
    )
    total = 0.0
    for r in res.results:
        out = np.asarray(r["out"], dtype=np.float64)  # [P, OUT_COLS]
        drow = np.asarray(r["dve"], dtype=np.float64).reshape(BL)  # [BL]
        # per-row sumexp: ACT partials (3 cols per row block) + DVE partial
        for rb in range(RB):
            se = out[:, rb * NT_ACT : (rb + 1) * NT_ACT].sum(axis=1)
            se = se + drow[rb * P : (rb + 1) * P]
            total += np.log(se).sum()
        total += out[:, ACC_COLS].sum()
    return np.float32(total / B)


# revision 12
# speedup vs baseline: 2.0778x; 1.1088x over previous
"""Multi-label masked-gather mean loss on 8 Trainium2 NeuronCores.

reference:
    logp = log_softmax(x, -1); per_sample = -sum_t(mask*logp[i, y[i,t]])/count_i
    loss = mean(per_sample)

Identity used (count_i > 0):
    per_sample_i = logsumexp(x_i) - sum_t w[i,t] * x[i, y[i,t]],  w = mask/count
    loss = (sum_i logsumexp(x_i) + sum_{i,t} wneg[i,t] * x[i,y[i,t]]) / B
with wneg = -w. Data-parallel over the batch: 4096 rows -> 512 rows/core.

v2: the exp+row-sum over the core's [512, 50257] shard is split across
engines so the scalar engine stops being the lone bottleneck:
  * ACT path (cols [0, CA)): row-major fp8 tiles, native Exp with fused
    accum_out per 128-row block (as v1, but fp8 halves DMA bytes).
  * DVE+PE path (cols [CA, C)): TRANSPOSED fp8 tiles [128 cols, 512 rows].
    DVE computes Schraudolph's bit-trick exp: int16(x*184.665 + B) viewed
    as bf16 IS approximately exp(x) (B tuned so the mean multiplicative
    bias over the fractional-exponent distribution is ~0). PE then sums
    along partitions (=columns) via an accumulating ones-matmul into
    PSUM[1, 512] = per-row partial sums.
Per-element exp error ~2-4% is iid across ~25k elements per row-half, so
row sumexp error is <<0.1%; loss tolerance is 2e-2.

Host sums ACT partials + DVE partials per row, takes log, adds the
gathered-label term, divides by B.
"""

import sys

sys.path.insert(0, "/opt/trn_rl_repo")

import math

import numpy as np

import concourse.bass as bass
import concourse.tile as tile
from concourse import bacc, mybir
from concourse import bass_utils

# Problem shape (hardcoded per contract)
B, C, T = 4096, 50257, 8
NCORES = 8
BL = B // NCORES  # 512 rows per core
P = 128
RB = BL // P      # 4 row blocks per core
GCOLS = BL * T // P      # 32: gathered elements per partition

# Column split: last N_CHUNK*128 columns go to the DVE+PE path (transposed
# layout), the first CA to the ACT path. Measured rates: ACT 0.93 ns/col,
# DVE convert 0.555 ns/col (the fp8->int16 convert hits the 2x path on HW);
# balance puts ~38% on ACT, both landing just under the fp8 DMA roofline.
N_CHUNK = 244
CT = N_CHUNK * P         # 31232 transposed cols
CA = C - CT              # 19025 ACT cols

# ACT path tile widths per row block (sum = CA). First tile shorter so ACT
# starts quickly; fp8 DMA runs ~2.4x ACT's rate so no long ramp needed.
ACT_WIDTHS = [2048, 8704, 8273]
assert sum(ACT_WIDTHS) == CA
MAXW = max(ACT_WIDTHS)
NT_ACT = len(ACT_WIDTHS)

ACC_COLS = RB * NT_ACT         # per-(rowblock, coltile) sumexp partials
OUT_COLS = ACC_COLS + 1        # + gather-dot col

# DVE+PE path geometry. The host pre-tiles the transposed shard into
# xt[p, chunk*512 + e] = x[e, CA + chunk*128 + p] so any slab of chunks is
# a plain 2D slice with contiguous per-partition lines (big descriptors).
K_SLAB = 16                    # chunks per DMA slab
SLABS = [K_SLAB] * (N_CHUNK // K_SLAB) + (
    [N_CHUNK % K_SLAB] if N_CHUNK % K_SLAB else []
)
N_SLAB = len(SLABS)
K_CONV = 4                     # chunks per DVE convert instruction
SLAB_F = K_SLAB * BL           # max slab free size (8192)
CONV_F = K_CONV * BL           # convert free size (2048)

# Schraudolph constants for bf16 bit patterns:
#   bits = x * 128*log2(e) + 128*(127 - c),  c = 0.0564298 zeroes the mean
#   multiplicative bias of the linear-mantissa approximation for f~U[0,1).
SCH_A = 128.0 * math.log2(math.e)          # 184.6650
SCH_B = 128.0 * (127.0 - 0.0564298)        # 16248.777

_f32 = mybir.dt.float32
_bf16 = mybir.dt.bfloat16
_fp8 = mybir.dt.float8e4
_i16 = mybir.dt.int16
_i32 = mybir.dt.int32

_compiled = None  # (nc, names) cache


def _build():
    nc = bacc.Bacc(
        "TRN2",
        target_bir_lowering=False,
        debug=False,
        enable_asserts=False,
        num_devices=NCORES,
    )
    x_t = nc.dram_tensor("x", [BL, C], _fp8, kind="ExternalInput")
    xt_t = nc.dram_tensor("xt", [P, N_CHUNK * BL], _fp8, kind="ExternalInput")
    idx_t = nc.dram_tensor("idx", [P, GCOLS], _i32, kind="ExternalInput")
    wneg_t = nc.dram_tensor("wneg", [P, GCOLS], _f32, kind="ExternalInput")
    # cols 0..ACC_COLS-1: ACT-path sumexp partials; col ACC_COLS: gather dot
    out_t = nc.dram_tensor("out", [P, OUT_COLS], _f32, kind="ExternalOutput")
    # DVE+PE path per-row partial sumexp
    dve_t = nc.dram_tensor("dve", [1, BL], _f32, kind="ExternalOutput")

    x = x_t.ap()
    xt = xt_t.ap()
    idx = idx_t.ap()
    wneg = wneg_t.ap()
    out = out_t.ap()
    dve = dve_t.ap()

    with tile.TileContext(nc) as tc:
        with (
            tc.tile_pool(name="xin", bufs=4) as xin_pool,
            tc.tile_pool(name="tin", bufs=4) as tin_pool,
            tc.tile_pool(name="tconv", bufs=4) as tconv_pool,
            tc.tile_pool(name="scratch", bufs=1) as scratch_pool,
            tc.tile_pool(name="stats", bufs=1) as stats_pool,
            tc.tile_pool(name="gather", bufs=1) as gather_pool,
            tc.psum_pool(name="psum", bufs=1) as psum_pool,
        ):
            # all ACT partials end up here and go out in one DMA
            acc = stats_pool.tile([P, OUT_COLS], _f32)
            # self-made zero bias for Exp. The scalar engine zeroes it itself
            # (memzero lowers to a Copy-activation, float bias allowed): the
            # gpsimd memset used before didn't run until ~6us in and stalled
            # ACT's first Exp until 11.4us.
            bias0 = stats_pool.tile([P, 1], _f32)
            nc.scalar.memzero(bias0[:])
            # ones weights for the PE partition-sum (DVE is idle early)
            ones_w = stats_pool.tile([P, 1], _bf16)
            nc.vector.memset(ones_w[:], 1.0)

            # exp output scratch: values are unused, only accum_out matters
            exp_scratch = scratch_pool.tile([P, MAXW], mybir.dt.float8e4)

            # PE accumulates per-row sums here across all chunks
            prow = psum_pool.tile([1, BL], _f32)

            # --- issue the first ACT tile so ACT starts ASAP ---
            def act_tile(rb, ti, c0, cw):
                rows = slice(rb * P, (rb + 1) * P)
                xtile = xin_pool.tile([P, MAXW], _fp8, tag="xt")
                nc.sync.dma_start(out=xtile[:, :cw], in_=x[rows, c0 : c0 + cw])
                nc.scalar.activation(
                    out=exp_scratch[:, :cw],
                    in_=xtile[:, :cw],
                    func=mybir.ActivationFunctionType.Exp,
                    bias=bias0[:, 0:1],
                    accum_out=acc[:, rb * NT_ACT + ti : rb * NT_ACT + ti + 1],
                )

            def dve_slab(s):
                # slab DMA: k chunks, plain 2D slice of the pre-tiled xt
                chunk0 = sum(SLABS[:s])
                k = SLABS[s]
                f_dram = chunk0 * BL
                tin = tin_pool.tile([P, SLAB_F], _fp8, tag="tin")
                nc.sync.dma_start(
                    out=tin[:, : k * BL], in_=xt[:, f_dram : f_dram + k * BL]
                )
                tcv = tconv_pool.tile([P, SLAB_F], _i16, tag="tconv")
                for ci in range((k + K_CONV - 1) // K_CONV):
                    f0 = ci * CONV_F
                    kc = min(K_CONV, k - ci * K_CONV)
                    nc.vector.tensor_scalar(
                        out=tcv[:, f0 : f0 + kc * BL],
                        in0=tin[:, f0 : f0 + kc * BL],
                        scalar1=SCH_A,
                        scalar2=SCH_B,
                        op0=mybir.AluOpType.mult,
                        op1=mybir.AluOpType.add,
                    )
                    ebits = tcv[:, f0 : f0 + kc * BL].bitcast(_bf16)
                    for j in range(kc):
                        chunk = chunk0 + ci * K_CONV + j
                        nc.tensor.matmul(
                            out=prow[0:1, :],
                            lhsT=ones_w[:],
                            rhs=ebits[:, j * BL : (j + 1) * BL],
                            start=(chunk == 0),
                            stop=(chunk == N_CHUNK - 1),
                        )

            # Interleave issuance: ACT rb0 ramp first, then alternate slabs
            # with remaining ACT tiles so both DMA streams stay ahead.
            def col_starts():
                st = []
                c0 = 0
                for w in ACT_WIDTHS:
                    st.append(c0)
                    c0 += w
                return st

            starts = col_starts()
            act_list = [
                (rb, ti, starts[ti], ACT_WIDTHS[ti])
                for rb in range(RB)
                for ti in range(NT_ACT)
            ]
            # issue order: 2 ACT tiles up front, then round-robin
            order = []
            ai, si = 0, 0
            order.append(("A", act_list[0]))
            ai = 1
            while ai < len(act_list) or si < N_SLAB:
                if si < N_SLAB:
                    order.append(("S", si))
                    si += 1
                if ai < len(act_list):
                    order.append(("A", act_list[ai]))
                    ai += 1
            # --- gather path DMAs (tiny; SWDGE on gpsimd, no deps) ---
            idx_tile = gather_pool.tile([P, GCOLS], _i32)
            nc.gpsimd.dma_start(out=idx_tile[:], in_=idx[:])
            w_tile = gather_pool.tile([P, GCOLS], _f32)
            nc.gpsimd.dma_start(out=w_tile[:], in_=wneg[:])
            g_tile = gather_pool.tile([P, GCOLS], _fp8)
            nc.gpsimd.indirect_dma_start(
                out=g_tile[:],
                out_offset=None,
                in_=x[:],
                in_offset=bass.IndirectOffsetOnAxis(ap=idx_tile[:], axis=1),
            )

            def gather_compute():
                # ~1.3us of DVE work; run it early (after slab 0's converts)
                # so the final out DMA doesn't wait on it at the tail.
                g32 = gather_pool.tile([P, GCOLS], _f32)
                nc.vector.tensor_copy(out=g32[:], in_=g_tile[:])
                gw = gather_pool.tile([P, GCOLS], _f32)
                nc.vector.tensor_tensor(
                    out=gw[:], in0=g32[:], in1=w_tile[:], op=mybir.AluOpType.mult
                )
                nc.vector.tensor_reduce(
                    out=acc[:, ACC_COLS : ACC_COLS + 1],
                    in_=gw[:],
                    axis=mybir.AxisListType.X,
                    op=mybir.AluOpType.add,
                )

            done_gather = False
            for kind, v in order:
                if kind == "A":
                    act_tile(*v)
                else:
                    dve_slab(v)
                    if not done_gather:
                        gather_compute()
                        done_gather = True

            # PSUM -> SBUF -> DRAM for the DVE per-row partials
            drow = stats_pool.tile([1, BL], _f32)
            nc.vector.tensor_copy(out=drow[:], in_=prow[0:1, :])
            nc.sync.dma_start(out=dve[:], in_=drow[:])

            # out via the scalar engine's HWDGE ring: no cross-engine hop
            # after the last ACT writes its accumulator column.
            nc.scalar.dma_start(out=out[:], in_=acc[:])

    nc.compile()
    return nc


def _get_compiled():
    global _compiled
    if _compiled is None:
        _compiled = _build()
    return _compiled


def _make_in_maps(x, y):
    import ml_dtypes

    fp8 = ml_dtypes.float8_e4m3
    x = np.asarray(x, dtype=np.float32)
    y = np.asarray(y)
    mask = y != -1
    cnt = mask.sum(axis=1)
    # rows with count 0 would be NaN in the reference; inputs never hit this
    w = np.where(mask, 1.0 / np.maximum(cnt, 1)[:, None], 0.0).astype(np.float32)
    wneg = -w
    safe = np.where(mask, y, 0).astype(np.int64)

    in_maps = []
    for m in range(NCORES):
        sl = slice(m * BL, (m + 1) * BL)
        xs = np.ascontiguousarray(x[sl].astype(fp8))
        # pre-tiled transpose: xt[p, chunk*BL + e] = x[e, CA + chunk*P + p]
        xts = np.ascontiguousarray(
            x[sl, CA:]
            .T.astype(fp8)
            .reshape(N_CHUNK, P, BL)
            .transpose(1, 0, 2)
            .reshape(P, N_CHUNK * BL)
        )
        flat = (
            np.arange(BL, dtype=np.int64)[:, None] * C + safe[sl]
        ).astype(np.int32)
        in_maps.append(
            {
                "x": xs,
                "xt": xts,
                "idx": np.ascontiguousarray(flat.reshape(P, GCOLS)),
                "wneg": np.ascontiguousarray(wneg[sl].reshape(P, GCOLS)),
            }
        )
    return in_maps


def kernel(**inputs) -> np.ndarray:
    x, y = inputs["x"], inputs["y"]
    nc = _get_compiled()
    in_maps = _make_in_maps(x, y)
    res = bass_utils.run_bass_kernel_spmd(
        nc, in_maps, core_ids=list(range(NCORES))
    )
    total = 0.0
    for r in res.results:
        out = np.asarray(r["out"], dtype=np.float64)  # [P, OUT_COLS]
        drow = np.asarray(r["dve"], dtype=np.float64).reshape(BL)  # [BL]
        # per-row sumexp: ACT partials (3 cols per row block) + DVE partial
        for rb in range(RB):
            se = out[:, rb * NT_ACT : (rb + 1) * NT_ACT].sum(axis=1)
            se = se + drow[rb * P : (rb + 1) * P]
            total += np.log(se).sum()
        total += out[:, ACC_COLS].sum()
    return np.float32(total / B)
